# revision 1
# baseline (speedup 1.0000x reference)
"""DHEL contrastive loss kernel for Trainium2 (8 NeuronCores, SPMD).

Math (reference):
  zhat = z / max(||z||, 1e-12) rowwise;  za = zhat[:8192], zp = zhat[8192:]
  sa_i = sum_j!=i exp(za_i . za_j / tau);  sp_i = sum_j!=i exp(zp_i . zp_j / tau)
  pdot_i = za_i . zp_i
  loss = mean_i( log sa_i + log sp_i - pdot_i / tau )

Sharding: row-parallel over the 8 cores; core c owns anchor rows
[c*1024,(c+1)*1024) and the matching positives. The host hands each core a
row-PERMUTED copy of z ("zn": my anchors, other anchors, my positives, other
positives) plus its contiguous transpose ("zc" = zn.T). The permutation makes
every device-side access pattern core-independent (one NEFF for all cores),
and is harmless because the exp row-sums are invariant to column order within
each half. The host mean over the returned 8x1024 per-row terms is the
unshard step.

Per-core device pipeline (ACT-bound: 2*8192^2/8 = 16.8M exp evaluations at
1 elem/lane/cycle = ~110us minimum on the scalar engine):
  - load zn tiles (natural) and zc column-groups (transposed) as fp32,
    processed in column chunks; the first group is split into 2x1024-wide
    chunks so the exp pipeline ramps early
  - row norms on compact natural tiles (DVE square+reduce, Quake-seed rsqrt
    with two Newton steps -- keeps the ACT stream pure exp, no table reloads)
  - the two ramp chunks normalize in natural layout and PE-TRANSPOSE into the
    matmul layout (PE is idle at startup); later chunks bounce the inverse
    norms through DRAM and return them partition-BROADCAST, making the
    transposed-layout normalize one DVE multiply: zt = bf16(zc * inv_bc)
  - main loop, one column chunk at a time: bf16 matmuls
    (128x128 @ 128x512 -> PSUM fp32) + ONE ACT exp over the whole PSUM chunk
    with accum_out giving the fused row-sum
  - subtract the exact self-similarity term exp(|bf16(zhat_i)|^2/tau)
    (bit-identical to the matmul diagonal), Ln both halves at the end (a
    mid-stream Ln costs two activation-table swaps), add the positive-pair
    dot, DMA out the 1024 per-row loss terms.
"""

import sys

if "/opt/trn_rl_repo" not in sys.path:
    sys.path.insert(0, "/opt/trn_rl_repo")

from contextlib import ExitStack

import numpy as np

import concourse.bass as bass  # noqa: F401  (bass types via bacc)
import concourse.tile as tile
from concourse import bacc, mybir
from concourse.bass_utils import run_bass_kernel_spmd
from concourse.masks import make_identity

P = 128            # SBUF partitions
D = 128            # embedding dim
M = 16384          # total rows of z
HALF = M // 2      # 8192 anchors / positives
NCORES = 8
Q = HALF // NCORES          # 1024 rows per core per half
RC = 8                      # row chunks per half (8 x 128 = 1024)
NSLOT = 5                   # accumulation slots per (half, row-chunk)
TAU = 0.3
SCALE = float(1.0 / TAU)

# column chunks: (global col0, width, half m, accum slot). The first group is
# split in two 1024-wide chunks to shorten the pipeline ramp; chunk 0 also
# holds this core's own anchor rows (the matmul lhsT), chunk 5 the positives.
CHUNKS = [
    (0, 1024, 0, 0),
    (1024, 1024, 0, 4),
    (2048, 2048, 0, 1),
    (4096, 2048, 0, 2),
    (6144, 2048, 0, 3),
    (8192, 2048, 1, 0),
    (10240, 2048, 1, 1),
    (12288, 2048, 1, 2),
    (14336, 2048, 1, 3),
]
LHS_CHUNK = {0: 0, 1: 5}

F32 = mybir.dt.float32
BF16 = mybir.dt.bfloat16
AF = mybir.ActivationFunctionType
OP = mybir.AluOpType
AX = mybir.AxisListType


def _rsqrt_dve(nc, pool, n2, inv, n_tiles):
    """inv = 1/sqrt(n2) on DVE only: Quake seed + 2 Newton steps.

    Seed bits = 0x5f3759df - bits(n2)/2, computed in float arithmetic via
    int<->float value casts (rel err <= ~3.5%); two Newton iterations
    y = y*(1.5 - 0.5*n2*y^2) land at ~5e-6 rel error. Stays entirely off the
    (bottleneck) scalar engine.
    """
    bf = pool.tile([P, n_tiles, 1], F32, tag="nrm_bf")
    nc.vector.tensor_copy(bf[:], n2[:].bitcast(mybir.dt.int32))
    nc.vector.tensor_scalar(
        bf[:], bf[:], -0.5, float(0x5F3759DF), op0=OP.mult, op1=OP.add
    )
    y = pool.tile([P, n_tiles, 1], F32, tag="nrm_y0")
    nc.vector.tensor_copy(y[:].bitcast(mybir.dt.int32), bf[:])
    t0 = pool.tile([P, n_tiles, 1], F32, tag="nrm_t0")
    t1 = pool.tile([P, n_tiles, 1], F32, tag="nrm_t1")
    yn = pool.tile([P, n_tiles, 1], F32, tag="nrm_yn")
    cur = y
    n_it = 2
    for it in range(n_it):
        nc.vector.tensor_tensor(t0[:], cur[:], cur[:], op=OP.mult)
        nc.vector.scalar_tensor_tensor(
            t1[:], t0[:], -0.5, n2[:], op0=OP.mult, op1=OP.mult
        )
        dst = inv if it == n_it - 1 else yn
        nc.vector.scalar_tensor_tensor(
            dst[:], t1[:], 1.5, cur[:], op0=OP.add, op1=OP.mult
        )
        cur = yn


def _build(ctx: ExitStack, tc: tile.TileContext, zn_ext, zc_ext, terms_ext):
    nc = tc.nc

    persist = ctx.enter_context(tc.tile_pool(name="persist", bufs=1))
    zt_pool = ctx.enter_context(tc.tile_pool(name="zt", bufs=1))
    work = ctx.enter_context(tc.tile_pool(name="work", bufs=3))
    invbc_pool = ctx.enter_context(tc.tile_pool(name="invbc", bufs=6))
    eout_pool = ctx.enter_context(tc.tile_pool(name="eout", bufs=2))
    psum_pool = ctx.enter_context(tc.tile_pool(name="psum", bufs=2, space="PSUM"))
    dram_pool = ctx.enter_context(tc.tile_pool(name="dscr", bufs=len(CHUNKS),
                                               space="DRAM"))

    zn_view = zn_ext.rearrange("(t p) d -> p t d", p=P)    # (128, 128, 128)

    # accumulators for the exp row-sum chunks: col = (m*RC + rc)*NSLOT + slot
    accs = persist.tile([P, 2 * RC * NSLOT], F32)
    nc.vector.memset(accs[:], 0.0)   # slot 4 of the positives half stays 0
    # bf16 normalized rows for this core's own rows (matmul-exact replicas)
    zbf_q = [
        persist.tile([P, RC, D], BF16, tag=f"zbfq{m}", name=f"zbfq{m}")
        for m in range(2)
    ]
    selfexp = [
        persist.tile([P, RC], F32, tag=f"selfexp{m}", name=f"selfexp{m}")
        for m in range(2)
    ]
    lnS = [
        persist.tile([P, RC], F32, tag=f"lnS{m}", name=f"lnS{m}")
        for m in range(2)
    ]
    pdot = persist.tile([P, RC, 1], F32)
    ident = persist.tile([P, P], BF16, name="ident")
    make_identity(nc, ident[:])
    zts = [
        zt_pool.tile([P, w], BF16, tag=f"zt{ci}", name=f"zt{ci}")
        for ci, (_, w, _, _) in enumerate(CHUNKS)
    ]

    def half_epilogue(m):
        """Row-sum -> subtract self-term -> Ln for half m (overlappable)."""
        SA = persist.tile([P, RC], F32, tag=f"SA{m}", name=f"SA{m}")
        nc.vector.tensor_reduce(
            SA[:],
            accs[:, m * RC * NSLOT : (m + 1) * RC * NSLOT].rearrange(
                "p (r c) -> p r c", c=NSLOT
            ),
            axis=AX.X, op=OP.add,
        )
        SAadj = persist.tile([P, RC], F32, tag=f"SAadj{m}", name=f"SAadj{m}")
        nc.vector.tensor_tensor(SAadj[:], SA[:], selfexp[m][:], op=OP.subtract)
        nc.scalar.activation(lnS[m][:], SAadj[:], AF.Ln)

    for ci, (col0, W, m, slot) in enumerate(CHUNKS):
        t0i = col0 // P
        ntl = W // P
        # natural tiles for norms (fp32, rotating), transposed column chunk
        znat = work.tile([P, ntl, D], F32, tag="znat")
        nc.sync.dma_start(znat[:], zn_view[:, t0i : t0i + ntl, :])
        if ci >= 2:
            zcol = work.tile([P, W], F32, tag="zcol")
            nc.sync.dma_start(zcol[:], zc_ext[:, col0 : col0 + W])

        # row norms of this chunk's rows
        sqg = work.tile([P, ntl, D], BF16, tag="sqg")
        n2 = work.tile([P, ntl, 1], F32, tag="n2")
        # square+reduce in halves: smaller DVE ops mean the scheduler's
        # greedy idle-fill steals stretch other chunks' serial chains less
        nh = ntl // 2
        for h in range(2):
            nc.vector.tensor_tensor(
                sqg[:, h * nh : (h + 1) * nh, :],
                znat[:, h * nh : (h + 1) * nh, :],
                znat[:, h * nh : (h + 1) * nh, :], op=OP.mult,
            )
            nc.vector.tensor_reduce(
                n2[:, h * nh : (h + 1) * nh, :],
                sqg[:, h * nh : (h + 1) * nh, :], axis=AX.X, op=OP.add,
            )
        inv = work.tile([P, ntl, 1], F32, tag="inv")
        _rsqrt_dve(nc, work, n2, inv, ntl)

        zt = zts[ci][:]
        if ci < 2:
            # ramp chunks: normalize in natural layout and PE-transpose into
            # zt (PE is idle here) -- skips the DRAM broadcast bounce and its
            # SWDGE descriptor serialization on the startup critical path.
            # Values are bit-identical to the bounce path (same f32 inputs,
            # same DVE rounding; the transpose moves bf16 verbatim).
            nat = zbf_q[0] if ci == 0 else work.tile([P, RC, D], BF16,
                                                     tag="nat1", name="nat1")
            nc.vector.tensor_tensor(
                nat[:], znat[:, 0:RC, :],
                inv[:, 0:RC, :].broadcast_to([P, RC, D]), op=OP.mult,
            )
            pst = psum_pool.tile([P, W], BF16, tag="ps", name=f"pst{ci}")
            for t in range(RC):
                nc.tensor.transpose(
                    pst[:, t * P : (t + 1) * P], nat[:, t, :], ident[:]
                )
            nc.vector.tensor_copy(zt, pst[:])
        else:
            # broadcast inv across partitions via a DRAM bounce: write the
            # norms in row order, read them back replicated to all partitions
            invd = dram_pool.tile([W], F32, tag="invd")
            nc.sync.dma_start(invd[:].rearrange("(t p) -> p t", p=P), inv[:])
            invbc = invbc_pool.tile([P, W], F32, tag="invbc")
            nc.gpsimd.dma_start(invbc[:], invd[:].partition_broadcast(P))
            # normalized bf16 column chunk (cast fused into the multiply)
            nc.vector.tensor_tensor(zt, zcol[:], invbc[:], op=OP.mult)

        # ---- main loop block for this column chunk ----
        lhsrc = zts[LHS_CHUNK[m]][:]
        for rc in range(RC):
            ps = psum_pool.tile([P, W], F32, tag="ps")
            for k in range(W // 512):
                nc.tensor.matmul(
                    ps[:, k * 512 : (k + 1) * 512],
                    lhsrc[:, rc * P : (rc + 1) * P],
                    zt[:, k * 512 : (k + 1) * 512],
                    start=True,
                    stop=True,
                )
            eo = eout_pool.tile([P, W], F32, tag="eo")
            col = (m * RC + rc) * NSLOT + slot
            nc.scalar.activation(
                eo[:], ps[:], AF.Exp, scale=SCALE,
                accum_out=accs[:, col : col + 1],
            )

        if ci in (0, 5):
            # my rows are tiles 0..7 of chunk 0 (anchors) / chunk 5
            # (positives): matmul-exact bf16 replica for pdot/self-sim
            # (chunk 0's was already built for the transpose path above)
            if ci == 5:
                nc.vector.tensor_tensor(
                    zbf_q[m][:],
                    znat[:, 0:RC, :],
                    inv[:, 0:RC, :].broadcast_to([P, RC, D]),
                    op=OP.mult,
                )
            sq2 = persist.tile([P, RC, D], F32, tag=f"sq2_{m}",
                               name=f"sq2_{m}")
            nc.vector.tensor_tensor(sq2[:], zbf_q[m][:], zbf_q[m][:],
                                    op=OP.mult)
            selfsq = persist.tile([P, RC, 1], F32, tag=f"selfsq{m}",
                                  name=f"selfsq{m}")
            nc.vector.tensor_reduce(selfsq[:], sq2[:], axis=AX.X, op=OP.add)
            nc.scalar.activation(
                selfexp[m][:].rearrange("p (r o) -> p r o", o=1),
                selfsq[:], AF.Exp, scale=SCALE,
            )
        if ci == 5:
            # pdot straight from the bf16 normalized vectors
            prodq = persist.tile([P, RC, D], F32, tag="prodq")
            nc.vector.tensor_tensor(prodq[:], zbf_q[0][:], zbf_q[1][:],
                                    op=OP.mult)
            nc.vector.tensor_reduce(pdot[:], prodq[:], axis=AX.X, op=OP.add)
    # both halves' logs at the end: a mid-stream Ln would force two
    # activation-table swaps (~2.6us) inside the exp stream
    half_epilogue(0)
    half_epilogue(1)

    # ---------------- final combine ----------------
    tsum = persist.tile([P, RC], F32)
    nc.vector.tensor_tensor(tsum[:], lnS[0][:], lnS[1][:], op=OP.add)
    terms = persist.tile([P, RC], F32)
    # terms = (pdot * -1/tau) + (ln sa + ln sp)
    nc.vector.scalar_tensor_tensor(
        terms[:], pdot[:].rearrange("p t o -> p (t o)"), -SCALE, tsum[:],
        op0=OP.mult, op1=OP.add,
    )
    nc.sync.dma_start(terms_ext.rearrange("t p -> p t"), terms[:])


def build_kernel() -> bass.Bass:
    nc = bacc.Bacc("TRN2", target_bir_lowering=False, debug=False,
                   num_devices=NCORES)
    zn_ext = nc.dram_tensor("zn", (M, D), F32, kind="ExternalInput").ap()
    zc_ext = nc.dram_tensor("zc", (D, M), F32, kind="ExternalInput").ap()
    terms_ext = nc.dram_tensor("terms", (RC, P), F32, kind="ExternalOutput").ap()
    with tile.TileContext(nc) as tc:
        with ExitStack() as ctx:
            _build(ctx, tc, zn_ext, zc_ext, terms_ext)
    nc.compile()
    return nc


_CACHE: dict = {}


def kernel(z, _trace: bool = False):
    z = np.ascontiguousarray(np.asarray(z, dtype=np.float32))
    assert z.shape == (M, D), z.shape
    if "nc" not in _CACHE:
        _CACHE["nc"] = build_kernel()
    nc = _CACHE["nc"]

    za, zp = z[:HALF], z[HALF:]
    in_maps = []
    for c in range(NCORES):
        sel = np.r_[c * Q : (c + 1) * Q, 0 : c * Q, (c + 1) * Q : HALF]
        zn = np.concatenate([za[sel], zp[sel]], axis=0)
        zc = np.ascontiguousarray(zn.T)
        in_maps.append({"zn": np.ascontiguousarray(zn), "zc": zc})

    res = run_bass_kernel_spmd(
        nc, in_maps, core_ids=list(range(NCORES)), trace=_trace
    )
    _CACHE["last_results"] = res
    terms = np.concatenate(
        [r["terms"].astype(np.float64).reshape(-1) for r in res.results]
    )
    return np.float32(terms.mean())



# revision 17
# speedup vs baseline: 1.9335x; 1.9335x over previous
"""DHEL contrastive loss kernel for Trainium2 (8 NeuronCores, SPMD).

Math (reference):
  zhat = z / max(||z||, 1e-12) rowwise;  za = zhat[:8192], zp = zhat[8192:]
  sa_i = sum_j!=i exp(za_i . za_j / tau);  sp_i = sum_j!=i exp(zp_i . zp_j / tau)
  loss = mean_i( log sa_i + log sp_i - (za_i . zp_i) / tau )

Strategy: the exp similarity matrices are SYMMETRIC, so each unordered pair
(i, j) is computed exactly once across the whole machine and contributes to
BOTH row-sums i and j:
  - row i side: the activation engine's fused accum_out while computing
    exp(X) for an X-block row-strip (free).
  - row j side: a column-sum matmul per 128x128 exp block
    (lhsT = E_block, rhs = ones -> psum[128, 1]), accumulated across
    contributing strips directly in one persistent PSUM bank. Stationary
    weight loads make this nearly free on the otherwise idle TensorE.
This halves the scalar-engine exp work -- the hard bottleneck -- from
16384^2/8 to ~8.52M evaluations per core.

Work split across cores: blocks of 128 rows per half (64 blocks). Core c
owns row-blocks i0 in {0, 8, ..., 56} (in ITS locally rotated copy) and for
each computes the pairs (i0, i0+d mod 64) for d = 0..31, plus d = 32 for
i0 in {0, 8, 16, 24}. The host hands core c a copy of the NORMALIZED
embeddings (the sharding hint's "all-gathered normalized embeddings"),
bf16, transposed, with rows rotated by 128*c within each half -- so a
single NEFF serves all cores while the union over cores covers every
unordered block pair exactly once.

Device per core: DMA zt (128 x 16384 bf16) -> per strip (16 = 8 row-blocks
x 2 halves) 3 psum chunks (<=1536 wide) of bf16 matmuls -> ONE exp
activation per chunk with accum_out row-sums -> per-block column-sum
matmuls into the mirror psum bank. Outputs: 48 accum slots + the mirror
bank. The host sums partials across cores, un-rotates, subtracts the exact
bf16-faithful self-term, and finishes with log/pdot/mean (O(M) work).
"""

import sys

if "/opt/trn_rl_repo" not in sys.path:
    sys.path.insert(0, "/opt/trn_rl_repo")

from contextlib import ExitStack

import numpy as np

import concourse.bass as bass  # noqa: F401
import concourse.tile as tile
from concourse import bacc, mybir
from concourse.bass_utils import run_bass_kernel_spmd

P = 128
D = 128
M = 16384
HALF = M // 2          # 8192 rows per half
NB = HALF // P         # 64 blocks of 128 rows per half
NCORES = 8
TAU = 0.3
SCALE = float(1.0 / TAU)

STRIP_I0 = [0, 8, 16, 24, 32, 40, 48, 56]   # local row-blocks owned per core
D32_I0 = {0, 8, 16, 24}                     # strips that also take d = 32
# chunk split of the 32/33-block arc (d offsets per chunk)
CHUNK_D = [list(range(0, 12)), list(range(12, 24)), list(range(24, 32))]
NSLOTS = 16 * 3                             # (half, i0) x chunk accum slots

F32 = mybir.dt.float32
BF16 = mybir.dt.bfloat16
AF = mybir.ActivationFunctionType
OP = mybir.AluOpType
AX = mybir.AxisListType

DMA_CHUNK = 2048


MIRW = 12                                   # mirror columns per chunk slot


def mirror_map():
    """(slot, packed-pos) -> (half, bj): where each mirror column lands."""
    out = []
    for h in range(2):
        for si, i0 in enumerate(STRIP_I0):
            for ci, ds in enumerate(CHUNK_D):
                dlist = list(ds)
                if ci == 2 and i0 in D32_I0:
                    dlist = dlist + [32]
                slot = (h * 8 + si) * 3 + ci
                k = 0
                for d in dlist:
                    if d == 0:
                        continue        # diagonal block: row-sum only
                    out.append((slot * MIRW + k, h, (i0 + d) % NB))
                    k += 1
    return out


def _build(ctx: ExitStack, tc: tile.TileContext, zt_ext, slots_ext, mirror_ext):
    nc = tc.nc
    persist = ctx.enter_context(tc.tile_pool(name="persist", bufs=1))
    e_pool = ctx.enter_context(tc.tile_pool(name="epool", bufs=3))
    xps_pool = ctx.enter_context(tc.tile_pool(name="xps", bufs=2, space="PSUM"))
    mir_pool = ctx.enter_context(tc.tile_pool(name="mir", bufs=2, space="PSUM"))

    ones = persist.tile([P, 1], BF16)
    nc.vector.memset(ones[:], 1.0)
    zt = persist.tile([P, M], BF16)              # normalized bf16, transposed
    slots = persist.tile([P, NSLOTS], F32)       # accum_out row-sum slots
    marc = persist.tile([P, NSLOTS * MIRW], F32)  # mirror col-sum staging
    nc.vector.memset(marc[:], 0.0)

    for j in range(M // DMA_CHUNK):
        c0 = j * DMA_CHUNK
        nc.sync.dma_start(zt[:, c0:c0 + DMA_CHUNK], zt_ext[:, c0:c0 + DMA_CHUNK])

    for h in range(2):
        hoff = h * HALF
        for si, i0 in enumerate(STRIP_I0):
            lhsT = zt[:, hoff + i0 * P: hoff + (i0 + 1) * P]
            for ci, ds in enumerate(CHUNK_D):
                dlist = list(ds)
                if ci == 2 and i0 in D32_I0:
                    dlist = dlist + [32]
                W = len(dlist) * P
                xps = xps_pool.tile([P, 1536], F32, tag="xps")
                pos = 0
                while pos < len(dlist):
                    run = 1
                    while (
                        run < 4
                        and pos + run < len(dlist)
                        and dlist[pos + run] == dlist[pos] + run
                        and (i0 + dlist[pos + run]) % NB
                        == (i0 + dlist[pos]) % NB + run
                    ):
                        run += 1
                    sb = (i0 + dlist[pos]) % NB
                    nc.tensor.matmul(
                        xps[:, pos * P:(pos + run) * P],
                        lhsT,
                        zt[:, hoff + sb * P: hoff + (sb + run) * P],
                        start=True, stop=True,
                    )
                    pos += run
                eo = e_pool.tile([P, 1536], BF16, tag="eo")
                slot = (h * 8 + si) * 3 + ci
                nc.scalar.activation(
                    eo[:, :W], xps[:, :W], AF.Exp, scale=SCALE,
                    accum_out=slots[:, slot:slot + 1],
                )
                scr = mir_pool.tile([P, MIRW], F32, tag="scr")
                k = 0
                for pos, d in enumerate(dlist):
                    if d == 0:
                        continue
                    nc.tensor.matmul(
                        scr[:, k:k + 1],
                        eo[:, pos * P:(pos + 1) * P],
                        ones[:], start=True, stop=True,
                    )
                    k += 1
                nc.vector.tensor_copy(
                    marc[:, slot * MIRW: slot * MIRW + k], scr[:, :k]
                )

    nc.sync.dma_start(slots_ext, slots[:])
    nc.sync.dma_start(mirror_ext, marc[:])


def build_kernel() -> bass.Bass:
    nc = bacc.Bacc("TRN2", target_bir_lowering=False, debug=False,
                   num_devices=NCORES)
    zt_ext = nc.dram_tensor("zt", (D, M), BF16, kind="ExternalInput").ap()
    slots_ext = nc.dram_tensor("slots", (P, NSLOTS), F32,
                               kind="ExternalOutput").ap()
    mirror_ext = nc.dram_tensor("mirror", (P, NSLOTS * MIRW), F32,
                                kind="ExternalOutput").ap()
    with tile.TileContext(nc) as tc:
        with ExitStack() as ctx:
            _build(ctx, tc, zt_ext, slots_ext, mirror_ext)
    nc.compile()
    return nc


_CACHE: dict = {}


def _normalize_bf16(z):
    """Host prep: f64 row-normalize then bf16 round (returns f32 values)."""
    import ml_dtypes

    zf = np.asarray(z, dtype=np.float64)
    zf = zf / np.maximum(np.linalg.norm(zf, axis=1, keepdims=True), 1e-12)
    return zf.astype(np.float32).astype(ml_dtypes.bfloat16)


def host_reduce(z, slots_all, mirror_all):
    """Combine per-core partials into the scalar loss (host, O(M) work)."""
    z = np.asarray(z, dtype=np.float32)
    mmap = mirror_map()
    S = np.zeros((2, HALF), dtype=np.float64)      # row sums incl. self term
    for c in range(NCORES):
        slots = slots_all[c].astype(np.float64).T    # (NSLOTS, P)
        mirror = mirror_all[c].astype(np.float64).T  # (NSLOTS*MIRW, P)
        for h in range(2):
            for si, i0 in enumerate(STRIP_I0):
                gb = (i0 + c) % NB
                rows = slice(gb * P, (gb + 1) * P)
                base = (h * 8 + si) * 3
                S[h, rows] += slots[base:base + 3].sum(axis=0)
        for col, h, bj in mmap:
            gb = (bj + c) % NB
            S[h, gb * P:(gb + 1) * P] += mirror[col]

    # self-term replica: exp(||zt_i||^2 / tau) from the same bf16 values the
    # device matmuls consume
    zt = _normalize_bf16(z).astype(np.float64)
    selfexp = np.exp((zt ** 2).sum(axis=1) * SCALE)
    Sa = S[0] - selfexp[:HALF]
    Sp = S[1] - selfexp[HALF:]

    # pdot from the true f32 inputs (exact math; device never computes it)
    zf = z.astype(np.float64)
    zf = zf / np.maximum(np.linalg.norm(zf, axis=1, keepdims=True), 1e-12)
    pdot = np.sum(zf[:HALF] * zf[HALF:], axis=1)

    terms = np.log(Sa) + np.log(Sp) - pdot * SCALE
    return np.float32(terms.mean())


def kernel(z, _trace: bool = False):
    z = np.ascontiguousarray(np.asarray(z, dtype=np.float32))
    assert z.shape == (M, D), z.shape
    if "nc" not in _CACHE:
        _CACHE["nc"] = build_kernel()
    nc = _CACHE["nc"]

    zt = _normalize_bf16(z)
    za, zp = zt[:HALF], zt[HALF:]
    in_maps = []
    for c in range(NCORES):
        zrot = np.concatenate(
            [np.roll(za, -P * c, axis=0), np.roll(zp, -P * c, axis=0)], axis=0
        )
        in_maps.append({"zt": np.ascontiguousarray(zrot.T)})

    res = run_bass_kernel_spmd(
        nc, in_maps, core_ids=list(range(NCORES)), trace=_trace
    )
    _CACHE["last_results"] = res
    slots_all = [r["slots"] for r in res.results]
    mirror_all = [r["mirror"] for r in res.results]
    return host_reduce(z, slots_all, mirror_all)


# revision 62
# speedup vs baseline: 2.1319x; 1.1026x over previous
"""DHEL contrastive loss kernel for Trainium2 (8 NeuronCores, SPMD).

Math (reference):
  zhat = z / max(||z||, 1e-12) rowwise;  za = zhat[:8192], zp = zhat[8192:]
  sa_i = sum_j!=i exp(za_i . za_j / tau);  sp_i = sum_j!=i exp(zp_i . zp_j / tau)
  loss = mean_i( log sa_i + log sp_i - (za_i . zp_i) / tau )

Strategy: the exp similarity matrices are SYMMETRIC, so each unordered pair
(i, j) is computed exactly once across the whole machine and contributes to
BOTH row-sums i and j:
  - row i side: the activation engine's fused accum_out while computing
    exp(X) for an X-block row-strip (free).
  - row j side: a column-sum matmul per 128x128 exp block
    (lhsT = E_block, rhs = ones -> psum[128, 1]), accumulated across
    contributing strips directly in one persistent PSUM bank. Stationary
    weight loads make this nearly free on the otherwise idle TensorE.
This halves the scalar-engine exp work -- the hard bottleneck -- from
16384^2/8 to ~8.52M evaluations per core.

Work split across cores: blocks of 128 rows per half (64 blocks). Core c
owns row-blocks i0 in {0, 8, ..., 56} (in ITS locally rotated copy) and for
each computes the pairs (i0, i0+d mod 64) for d = 0..31, plus d = 32 for
i0 in {0, 8, 16, 24}. The host hands core c a copy of the NORMALIZED
embeddings (the sharding hint's "all-gathered normalized embeddings"),
bf16, transposed, with rows rotated by 128*c within each half -- so a
single NEFF serves all cores while the union over cores covers every
unordered block pair exactly once.

Device per core: DMA zt (128 x 16384 bf16) -> per strip (16 = 8 row-blocks
x 2 halves) 3 psum chunks (<=1536 wide) of bf16 matmuls -> ONE exp
activation per chunk with accum_out row-sums -> per-block column-sum
matmuls into the mirror psum bank. Outputs: 48 accum slots + the mirror
bank. The host sums partials across cores, un-rotates, subtracts the exact
bf16-faithful self-term, and finishes with log/pdot/mean (O(M) work).
"""

import sys

if "/opt/trn_rl_repo" not in sys.path:
    sys.path.insert(0, "/opt/trn_rl_repo")

from contextlib import ExitStack

import numpy as np

import concourse.bass as bass  # noqa: F401
import concourse.tile as tile
from concourse import bacc, mybir
from concourse.bass_utils import run_bass_kernel_spmd

P = 128
D = 128
M = 16384
HALF = M // 2          # 8192 rows per half
NB = HALF // P         # 64 blocks of 128 rows per half
NCORES = 8
TAU = 0.3
SCALE = float(1.0 / TAU)

STRIP_I0 = [0, 8, 16, 24, 32, 40, 48, 56]   # local row-blocks owned per core
D32_I0 = [0, 8, 16, 24]                     # blocks pairing with +32 (d32)
NCHUNK = 3                                  # chunks per strip
NSLOTS = 16 * NCHUNK + 24                   # strip slots + d31/d32 unit slots


def strip_chunks(i0):
    """d-offset lists for the 3 chunks of a strip: two 1536-wide ACT chunks
    (psum arenas A/B), one 896-wide DVE Schraudolph chunk (arena C, whose
    tail holds the strip's mirror col-sum scratch). The d=31 and d=32 pairs
    are handled by batched per-half DVE units in the A arena."""
    return [list(range(0, 12)), list(range(12, 24)), list(range(24, 31))]

F32 = mybir.dt.float32
I32 = mybir.dt.int32
BF16 = mybir.dt.bfloat16
AF = mybir.ActivationFunctionType
OP = mybir.AluOpType
AX = mybir.AxisListType

DMA_CHUNK = 1024

# Schraudolph fast-exp constants: bits(exp(s)) ~= s * 2^23/ln2 + (127<<23) - C
# with C tuned so the mean relative error over the similarity distribution is
# ~+2e-4 (rms 1.8%, max 3.8% -- the 8k-term row sums average the noise away).
SCH_A = float((1 << 23) / np.log(2.0)) * SCALE
SCH_B = float(127 << 23) - 475000.0


MIRW = 12                                   # mirror columns per chunk slot
CHUNK_KIND = ["act", "act", "dvec"]
MARC_X = 16 * NCHUNK * MIRW                 # marc cols where d31/d32 land
# batched-unit slot/marc bases per half: d31 gets 8 slots, d32 gets 4
SLOT_D31 = [48, 60]
SLOT_D32 = [56, 68]
MARC_D31 = [MARC_X, MARC_X + 12]
MARC_D32 = [MARC_X + 8, MARC_X + 20]


def mirror_map():
    """(marc col, half, bj): where each mirror column lands."""
    out = []
    for h in range(2):
        for si, i0 in enumerate(STRIP_I0):
            for ci, dlist in enumerate(strip_chunks(i0)):
                slot = (h * 8 + si) * NCHUNK + ci
                k = 0
                for d in dlist:
                    if d == 0:
                        continue        # diagonal block: row-sum only
                    out.append((slot * MIRW + k, h, (i0 + d) % NB))
                    k += 1
        for idx, i0 in enumerate(STRIP_I0):
            out.append((MARC_D31[h] + idx, h, (i0 + 31) % NB))
        for idx, i0 in enumerate(D32_I0):
            out.append((MARC_D32[h] + idx, h, i0 + 32))
    return out


def _build(ctx: ExitStack, tc: tile.TileContext, zt_ext, slots_ext, mirror_ext):
    nc = tc.nc
    persist = ctx.enter_context(tc.tile_pool(name="persist", bufs=1))
    e_pool = ctx.enter_context(tc.tile_pool(name="epool", bufs=6))
    ei_pool = ctx.enter_context(tc.tile_pool(name="eipool", bufs=6))
    fold_pool = ctx.enter_context(tc.tile_pool(name="fold", bufs=6))
    # fixed psum arenas, no rotation: ci0+mini share A, ci1 uses B, ci2+scr
    # share C -- 3+3+2 banks. Cross-strip reuse is WAR-chained per tag; the
    # consumer layout guarantees each fill's WAR target finished >1 unit ago.
    xps_pool = ctx.enter_context(tc.tile_pool(name="xpsAB", bufs=1, space="PSUM"))

    ones = persist.tile([P, 1], BF16)
    nc.vector.memset(ones[:], 1.0)
    ones_f = persist.tile([P, 1], F32)
    nc.vector.memset(ones_f[:], 1.0)
    zt = persist.tile([P, M], BF16)              # normalized bf16, transposed
    slots = persist.tile([P, NSLOTS], F32)       # accum_out row-sum slots
    marc = persist.tile([P, NSLOTS * MIRW], F32)  # mirror col-sum staging
    nc.vector.memset(marc[:], 0.0)

    for j in range(M // DMA_CHUNK):
        c0 = j * DMA_CHUNK
        nc.sync.dma_start(zt[:, c0:c0 + DMA_CHUNK], zt_ext[:, c0:c0 + DMA_CHUNK])

    u = 0
    pending = []        # (unit, emit_fn): colsums/copies deferred 2 units

    def flush_pending(upto):
        while pending and pending[0][0] <= upto:
            pending.pop(0)[1]()

    def emit_batched(h, pairs, slot0, marc0):
        """Batched DVE unit: n (bi, bj) block pairs, Schraudolph, one 3-D
        reduce into n slots, colsums past the X data. Lives in the C arena,
        whose chain has no ACT dependency -- the insertion only consumes
        DVE/Pool/PE slack."""
        hoff = h * HALF
        n = len(pairs)
        W = n * P
        xpa = xps_pool.tile([P, 896 + NCHUNK * MIRW], F32, tag="xpc")
        for idx, (bi, bj) in enumerate(pairs):
            nc.tensor.matmul(
                xpa[:, idx * P:(idx + 1) * P],
                zt[:, hoff + bi * P: hoff + (bi + 1) * P],
                zt[:, hoff + bj * P: hoff + (bj + 1) * P],
                start=True, stop=True,
            )
        ei = ei_pool.tile([P, 1024], I32, tag="ei")
        nc.vector.tensor_scalar(
            ei[:, :W], xpa[:, :W], SCH_A, SCH_B, op0=OP.mult, op1=OP.add
        )
        ef = ei[:].bitcast(F32)
        nc.vector.tensor_reduce(
            slots[:, slot0:slot0 + n].rearrange("p (a b) -> p a b", b=1),
            ef[:, :W].rearrange("p (a d) -> p a d", d=P),
            axis=AX.X, op=OP.add,
        )
        for idx in range(n):
            nc.tensor.matmul(
                xpa[:, W + idx: W + idx + 1],
                ef[:, idx * P:(idx + 1) * P],
                ones_f[:], start=True, stop=True,
            )
        nc.vector.tensor_copy(
            marc[:, marc0:marc0 + n], xpa[:, W:W + n]
        )

    for h in range(2):
        hoff = h * HALF
        for si, i0 in enumerate(STRIP_I0):
            lhsT = zt[:, hoff + i0 * P: hoff + (i0 + 1) * P]
            xpc = xps_pool.tile([P, 896 + NCHUNK * MIRW], F32, tag="xpc")
            for ci, dlist in enumerate(strip_chunks(i0)):
                W = len(dlist) * P
                xps = xpc if ci == 2 else xps_pool.tile(
                    [P, 1536], F32, tag="xpsA" if ci == 0 else "xpsB")
                pos = 0
                while pos < len(dlist):
                    run = 1
                    while (
                        run < 4
                        and pos + run < len(dlist)
                        and dlist[pos + run] == dlist[pos] + run
                        and (i0 + dlist[pos + run]) % NB
                        == (i0 + dlist[pos]) % NB + run
                    ):
                        run += 1
                    sb = (i0 + dlist[pos]) % NB
                    nc.tensor.matmul(
                        xps[:, pos * P:(pos + run) * P],
                        lhsT,
                        zt[:, hoff + sb * P: hoff + (sb + run) * P],
                        start=True, stop=True,
                    )
                    pos += run
                slot = (h * 8 + si) * NCHUNK + ci
                kind = CHUNK_KIND[ci]
                if h == 1 and si == 7 and ci == 2:
                    kind = "act"     # keep the tail on ACT so DVE/Pool drain
                u += 1
                if kind == "act":
                    eo = e_pool.tile([P, 1536], BF16, tag="eo")
                    nc.scalar.activation(
                        eo[:, :W], xps[:, :W], AF.Exp, scale=SCALE,
                        accum_out=slots[:, slot:slot + 1],
                    )
                    ef = eo
                else:
                    # Schraudolph fast exp on DVE: bits = X*(A*scale) + B as
                    # int32, reinterpreted as f32. Rowsum: 2-level tree fold
                    # on GpSimd (no PSUM port; it reads the SBUF bits tile),
                    # then a short DVE reduce.
                    ei = ei_pool.tile([P, 1024], I32, tag="ei")
                    nc.vector.tensor_scalar(
                        ei[:, :W], xps[:, :W], SCH_A, SCH_B,
                        op0=OP.mult, op1=OP.add,
                    )
                    ef = ei[:].bitcast(F32)
                    q = W // 4
                    f1 = fold_pool.tile([P, 512], F32, tag="f1")
                    nc.gpsimd.tensor_tensor(
                        f1[:, :2 * q], ef[:, :2 * q], ef[:, 2 * q:4 * q],
                        op=OP.add,
                    )
                    f2 = fold_pool.tile([P, 256], F32, tag="f2")
                    nc.gpsimd.tensor_tensor(
                        f2[:, :q], f1[:, :q], f1[:, q:2 * q], op=OP.add
                    )

                    def emit_reduce(f2=f2, q=q, slot=slot):
                        nc.vector.tensor_reduce(
                            slots[:, slot:slot + 1], f2[:, :q], axis=AX.X,
                            op=OP.add,
                        )

                    pending.append((u - 1, emit_reduce))
                # mirror colsums, padded to MIRW with dummy repeats of pos 0.
                # The strip's scratch is the consumed head of the C arena
                # (WAR on the chunk-2 consumer orders it safely).
                cols = [pos for pos, d in enumerate(dlist) if d != 0]
                cols = cols + [0] * (MIRW - len(cols))
                o = ones if kind == "act" else ones_f

                def emit_mirror(xpc=xpc, ci=ci, cols=cols, ef=ef, o=o):
                    for k, pos in enumerate(cols):
                        col = 896 + ci * MIRW + k
                        nc.tensor.matmul(
                            xpc[:, col:col + 1],
                            ef[:, pos * P:(pos + 1) * P],
                            o[:], start=True, stop=True,
                        )

                pending.append((u - 1, emit_mirror))
                flush_pending(u - 3)
            sbase = (h * 8 + si) * NCHUNK * MIRW

            def emit_copy(xpc=xpc, sbase=sbase):
                nc.vector.tensor_copy(
                    marc[:, sbase:sbase + NCHUNK * MIRW],
                    xpc[:, 896:896 + NCHUNK * MIRW],
                )

            pending.append((u - 1, emit_copy))
            if si == 1:
                emit_batched(h, [(b, (b + 31) % NB) for b in STRIP_I0[:4]],
                             SLOT_D31[h], MARC_D31[h])
            elif si == 3:
                emit_batched(h, [(b, (b + 31) % NB) for b in STRIP_I0[4:]],
                             SLOT_D31[h] + 4, MARC_D31[h] + 4)
            elif si == 5:
                emit_batched(h, [(b, b + 32) for b in D32_I0],
                             SLOT_D32[h], MARC_D32[h])

    flush_pending(10 ** 9)
    nc.sync.dma_start(slots_ext, slots[:])
    nc.sync.dma_start(mirror_ext, marc[:])


def build_kernel() -> bass.Bass:
    nc = bacc.Bacc("TRN2", target_bir_lowering=False, debug=False,
                   num_devices=NCORES)
    zt_ext = nc.dram_tensor("zt", (D, M), BF16, kind="ExternalInput").ap()
    slots_ext = nc.dram_tensor("slots", (P, NSLOTS), F32,
                               kind="ExternalOutput").ap()
    mirror_ext = nc.dram_tensor("mirror", (P, NSLOTS * MIRW), F32,
                                kind="ExternalOutput").ap()
    with tile.TileContext(nc) as tc:
        with ExitStack() as ctx:
            _build(ctx, tc, zt_ext, slots_ext, mirror_ext)
    nc.compile()
    return nc


_CACHE: dict = {}


def _normalize_bf16(z):
    """Host prep: f64 row-normalize then bf16 round (returns f32 values)."""
    import ml_dtypes

    zf = np.asarray(z, dtype=np.float64)
    zf = zf / np.maximum(np.linalg.norm(zf, axis=1, keepdims=True), 1e-12)
    return zf.astype(np.float32).astype(ml_dtypes.bfloat16)


def host_reduce(z, slots_all, mirror_all):
    """Combine per-core partials into the scalar loss (host, O(M) work)."""
    z = np.asarray(z, dtype=np.float32)
    mmap = mirror_map()
    S = np.zeros((2, HALF), dtype=np.float64)      # row sums incl. self term
    for c in range(NCORES):
        slots = slots_all[c].astype(np.float64).T    # (NSLOTS, P)
        mirror = mirror_all[c].astype(np.float64).T  # (NSLOTS*MIRW, P)
        for h in range(2):
            for si, i0 in enumerate(STRIP_I0):
                gb = (i0 + c) % NB
                rows = slice(gb * P, (gb + 1) * P)
                base = (h * 8 + si) * NCHUNK
                S[h, rows] += slots[base:base + NCHUNK].sum(axis=0)
            for idx, i0 in enumerate(STRIP_I0):
                gb = (i0 + c) % NB
                rows = slice(gb * P, (gb + 1) * P)
                S[h, rows] += slots[SLOT_D31[h] + idx]
            for idx, i0 in enumerate(D32_I0):
                gb = (i0 + c) % NB
                rows = slice(gb * P, (gb + 1) * P)
                S[h, rows] += slots[SLOT_D32[h] + idx]
        for col, h, bj in mmap:
            gb = (bj + c) % NB
            S[h, gb * P:(gb + 1) * P] += mirror[col]

    # self-term replica: exp(||zt_i||^2 / tau) from the same bf16 values the
    # device matmuls consume
    zt = _normalize_bf16(z).astype(np.float64)
    selfexp = np.exp((zt ** 2).sum(axis=1) * SCALE)
    Sa = S[0] - selfexp[:HALF]
    Sp = S[1] - selfexp[HALF:]

    # pdot from the true f32 inputs (exact math; device never computes it)
    zf = z.astype(np.float64)
    zf = zf / np.maximum(np.linalg.norm(zf, axis=1, keepdims=True), 1e-12)
    pdot = np.sum(zf[:HALF] * zf[HALF:], axis=1)

    terms = np.log(Sa) + np.log(Sp) - pdot * SCALE
    return np.float32(terms.mean())


def kernel(z, _trace: bool = False):
    z = np.ascontiguousarray(np.asarray(z, dtype=np.float32))
    assert z.shape == (M, D), z.shape
    if "nc" not in _CACHE:
        _CACHE["nc"] = build_kernel()
    nc = _CACHE["nc"]

    zt = _normalize_bf16(z)
    za, zp = zt[:HALF], zt[HALF:]
    in_maps = []
    for c in range(NCORES):
        zrot = np.concatenate(
            [np.roll(za, -P * c, axis=0), np.roll(zp, -P * c, axis=0)], axis=0
        )
        in_maps.append({"zt": np.ascontiguousarray(zrot.T)})

    res = run_bass_kernel_spmd(
        nc, in_maps, core_ids=list(range(NCORES)), trace=_trace
    )
    _CACHE["last_results"] = res
    slots_all = [r["slots"] for r in res.results]
    mirror_all = [r["mirror"] for r in res.results]
    return host_reduce(z, slots_all, mirror_all)


# revision 85
# speedup vs baseline: 2.1653x; 1.0157x over previous
"""DHEL contrastive loss kernel for Trainium2 (8 NeuronCores, SPMD).

Math (reference):
  zhat = z / max(||z||, 1e-12) rowwise;  za = zhat[:8192], zp = zhat[8192:]
  sa_i = sum_j!=i exp(za_i . za_j / tau);  sp_i = sum_j!=i exp(zp_i . zp_j / tau)
  loss = mean_i( log sa_i + log sp_i - (za_i . zp_i) / tau )

Strategy: the exp similarity matrices are SYMMETRIC, so each unordered pair
(i, j) is computed exactly once across the whole machine and contributes to
BOTH row-sums i and j:
  - row i side: the activation engine's fused accum_out while computing
    exp(X) for an X-block row-strip (free).
  - row j side: a column-sum matmul per 128x128 exp block
    (lhsT = E_block, rhs = ones -> psum[128, 1]), accumulated across
    contributing strips directly in one persistent PSUM bank. Stationary
    weight loads make this nearly free on the otherwise idle TensorE.
This halves the scalar-engine exp work -- the hard bottleneck -- from
16384^2/8 to ~8.52M evaluations per core.

Work split across cores: blocks of 128 rows per half (64 blocks). Core c
owns row-blocks i0 in {0, 8, ..., 56} (in ITS locally rotated copy) and for
each computes the pairs (i0, i0+d mod 64) for d = 0..31, plus d = 32 for
i0 in {0, 8, 16, 24}. The host hands core c a copy of the NORMALIZED
embeddings (the sharding hint's "all-gathered normalized embeddings"),
bf16, transposed, with rows rotated by 128*c within each half -- so a
single NEFF serves all cores while the union over cores covers every
unordered block pair exactly once.

Device per core: DMA zt (128 x 16384 bf16) -> per strip (16 = 8 row-blocks
x 2 halves) 3 psum chunks (<=1536 wide) of bf16 matmuls -> ONE exp
activation per chunk with accum_out row-sums -> per-block column-sum
matmuls into the mirror psum bank. Outputs: 48 accum slots + the mirror
bank. The host sums partials across cores, un-rotates, subtracts the exact
bf16-faithful self-term, and finishes with log/pdot/mean (O(M) work).
"""

import sys

if "/opt/trn_rl_repo" not in sys.path:
    sys.path.insert(0, "/opt/trn_rl_repo")

from contextlib import ExitStack

import numpy as np

import concourse.bass as bass  # noqa: F401
import concourse.tile as tile
from concourse import bacc, mybir
from concourse.bass_utils import run_bass_kernel_spmd

P = 128
D = 128
M = 16384
HALF = M // 2          # 8192 rows per half
NB = HALF // P         # 64 blocks of 128 rows per half
NCORES = 8
TAU = 0.3
SCALE = float(1.0 / TAU)

STRIP_I0 = [0, 8, 16, 24, 32, 40, 48, 56]   # local row-blocks owned per core
D32_I0 = [0, 8, 16, 24]                     # blocks pairing with +32 (d32)
NCHUNK = 3                                  # chunks per strip
NSLOTS = 16 * NCHUNK + 24                   # strip slots + d31/d32 unit slots


def strip_chunks(i0):
    """d-offset lists for the 3 chunks of a strip: two 1536-wide ACT chunks
    (psum arenas A/B), one 896-wide DVE Schraudolph chunk (arena C, whose
    tail holds the strip's mirror col-sum scratch). The d=31 and d=32 pairs
    are handled by batched per-half DVE units in the A arena."""
    return [list(range(0, 12)), list(range(12, 24)), list(range(24, 31))]

F32 = mybir.dt.float32
I32 = mybir.dt.int32
BF16 = mybir.dt.bfloat16
AF = mybir.ActivationFunctionType
OP = mybir.AluOpType
AX = mybir.AxisListType

DMA_CHUNK = 1024

# Schraudolph fast-exp constants: bits(exp(s)) ~= s * 2^23/ln2 + (127<<23) - C
# with C tuned so the mean relative error over the similarity distribution is
# ~+2e-4 (rms 1.8%, max 3.8% -- the 8k-term row sums average the noise away).
SCH_A = float((1 << 23) / np.log(2.0)) * SCALE
SCH_B = float(127 << 23) - 475000.0


MIRW = 12                                   # mirror columns per chunk slot
CHUNK_KIND = ["act", "act", "dvec"]
MARC_X = 16 * NCHUNK * MIRW                 # marc cols where d31/d32 land
# batched-unit slot/marc bases per half: d31 gets 8 slots, d32 gets 4
SLOT_D31 = [48, 60]
SLOT_D32 = [56, 68]
MARC_D31 = [MARC_X, MARC_X + 12]
MARC_D32 = [MARC_X + 8, MARC_X + 20]


def mirror_map():
    """(marc col, half, bj): where each mirror column lands."""
    out = []
    for h in range(2):
        for si, i0 in enumerate(STRIP_I0):
            for ci, dlist in enumerate(strip_chunks(i0)):
                slot = (h * 8 + si) * NCHUNK + ci
                k = 0
                for d in dlist:
                    if d == 0:
                        continue        # diagonal block: row-sum only
                    out.append((slot * MIRW + k, h, (i0 + d) % NB))
                    k += 1
        for idx, i0 in enumerate(STRIP_I0):
            out.append((MARC_D31[h] + idx, h, (i0 + 31) % NB))
        for idx, i0 in enumerate(D32_I0):
            out.append((MARC_D32[h] + idx, h, i0 + 32))
    return out


def _build(ctx: ExitStack, tc: tile.TileContext, zt_ext, slots_ext, mirror_ext):
    nc = tc.nc
    persist = ctx.enter_context(tc.tile_pool(name="persist", bufs=1))
    e_pool = ctx.enter_context(tc.tile_pool(name="epool", bufs=6))
    ei_pool = ctx.enter_context(tc.tile_pool(name="eipool", bufs=6))
    fold_pool = ctx.enter_context(tc.tile_pool(name="fold", bufs=6))
    # fixed psum arenas, no rotation: ci0+mini share A, ci1 uses B, ci2+scr
    # share C -- 3+3+2 banks. Cross-strip reuse is WAR-chained per tag; the
    # consumer layout guarantees each fill's WAR target finished >1 unit ago.
    xps_pool = ctx.enter_context(tc.tile_pool(name="xpsAB", bufs=1, space="PSUM"))

    ones = persist.tile([P, 1], BF16)
    nc.vector.memset(ones[:], 1.0)
    ones_f = persist.tile([P, 1], F32)
    nc.vector.memset(ones_f[:], 1.0)
    zt = persist.tile([P, M], BF16)              # normalized bf16, transposed
    slots = persist.tile([P, NSLOTS], F32)       # accum_out row-sum slots
    marc = persist.tile([P, NSLOTS * MIRW], F32)  # mirror col-sum staging
    nc.vector.memset(marc[:], 0.0)

    sizes = [512] * 4 + [1024] * 14     # small first chunks: faster ramp
    c0 = 0
    for j, w in enumerate(sizes):
        nc.sync.dma_start(zt[:, c0:c0 + w], zt_ext[:, c0:c0 + w])
        c0 += w

    u = 0
    pending = []        # (unit, emit_fn): colsums/copies deferred 2 units

    def flush_pending(upto):
        while pending and pending[0][0] <= upto:
            pending.pop(0)[1]()

    def emit_batched(h, pairs, slot0, marc0):
        """Batched DVE unit: n (bi, bj) block pairs, Schraudolph, one 3-D
        reduce into n slots, colsums past the X data. Lives in the C arena,
        whose chain has no ACT dependency -- the insertion only consumes
        DVE/Pool/PE slack."""
        hoff = h * HALF
        n = len(pairs)
        W = n * P
        xpa = xps_pool.tile([P, 896 + 2 * NCHUNK * MIRW], F32, tag="xpc")
        for idx, (bi, bj) in enumerate(pairs):
            nc.tensor.matmul(
                xpa[:, idx * P:(idx + 1) * P],
                zt[:, hoff + bi * P: hoff + (bi + 1) * P],
                zt[:, hoff + bj * P: hoff + (bj + 1) * P],
                start=True, stop=True,
            )
        ei = ei_pool.tile([P, 1024], I32, tag="ei")
        nc.vector.tensor_scalar(
            ei[:, :W], xpa[:, :W], SCH_A, SCH_B, op0=OP.mult, op1=OP.add
        )
        ef = ei[:].bitcast(F32)
        nc.vector.tensor_reduce(
            slots[:, slot0:slot0 + n].rearrange("p (a b) -> p a b", b=1),
            ef[:, :W].rearrange("p (a d) -> p a d", d=P),
            axis=AX.X, op=OP.add,
        )
        for idx in range(n):
            nc.tensor.matmul(
                xpa[:, W + idx: W + idx + 1],
                ef[:, idx * P:(idx + 1) * P],
                ones_f[:], start=True, stop=True,
            )
        nc.vector.tensor_copy(
            marc[:, marc0:marc0 + n], xpa[:, W:W + n]
        )

    for h in range(2):
        hoff = h * HALF
        for si, i0 in enumerate(STRIP_I0):
            lhsT = zt[:, hoff + i0 * P: hoff + (i0 + 1) * P]
            xpc = xps_pool.tile([P, 896 + 2 * NCHUNK * MIRW], F32, tag="xpc")
            # alternate scratch region by strip parity so this strip's
            # colsums WAW-chain to the copy from 2 strips ago, not 1
            scr0 = 896 + ((h * 8 + si) % 2) * NCHUNK * MIRW
            chunks = strip_chunks(i0)
            for ci in (0, 1, 2):
                dlist = chunks[ci]
                W = len(dlist) * P
                xps = xpc if ci == 2 else xps_pool.tile(
                    [P, 1536], F32, tag="xpsA" if ci == 0 else "xpsB")
                pos = 0
                if True:
                    while pos < len(dlist):
                        run = 1
                        while (
                            run < 4
                            and pos + run < len(dlist)
                            and dlist[pos + run] == dlist[pos] + run
                            and (i0 + dlist[pos + run]) % NB
                            == (i0 + dlist[pos]) % NB + run
                        ):
                            run += 1
                        sb = (i0 + dlist[pos]) % NB
                        nc.tensor.matmul(
                            xps[:, pos * P:(pos + run) * P],
                            lhsT,
                            zt[:, hoff + sb * P: hoff + (sb + run) * P],
                            start=True, stop=True,
                        ).annotate(f"fill_h{h}s{si}c{ci}p{pos}")
                        pos += run
                slot = (h * 8 + si) * NCHUNK + ci
                kind = CHUNK_KIND[ci]
                if h == 1 and si == 7 and ci == 2:
                    kind = "act"     # keep the tail on ACT so DVE/Pool drain
                u += 1
                if kind == "act":
                    eo = e_pool.tile([P, 1536], BF16, tag="eo")
                    nc.scalar.activation(
                        eo[:, :W], xps[:, :W], AF.Exp, scale=SCALE,
                        accum_out=slots[:, slot:slot + 1],
                    ).annotate(f"ACT_h{h}s{si}c{ci}")
                    ef = eo
                else:
                    # Schraudolph fast exp on DVE: bits = X*(A*scale) + B as
                    # int32, reinterpreted as f32. Rowsum: 2-level tree fold
                    # on GpSimd (no PSUM port; it reads the SBUF bits tile),
                    # then a short DVE reduce.
                    ei = ei_pool.tile([P, 1536], I32, tag="ei")
                    nc.vector.tensor_scalar(
                        ei[:, :W], xps[:, :W], SCH_A, SCH_B,
                        op0=OP.mult, op1=OP.add,
                    ).annotate(f"sch_h{h}s{si}c{ci}")
                    ef = ei[:].bitcast(F32)
                    q = W // 4
                    f1 = fold_pool.tile([P, 768], F32, tag="f1")
                    nc.gpsimd.tensor_tensor(
                        f1[:, :2 * q], ef[:, :2 * q], ef[:, 2 * q:4 * q],
                        op=OP.add,
                    )
                    f2 = fold_pool.tile([P, 384], F32, tag="f2")
                    nc.gpsimd.tensor_tensor(
                        f2[:, :q], f1[:, :q], f1[:, q:2 * q], op=OP.add
                    )

                    def emit_reduce(f2=f2, q=q, slot=slot):
                        nc.vector.tensor_reduce(
                            slots[:, slot:slot + 1], f2[:, :q], axis=AX.X,
                            op=OP.add,
                        )

                    pending.append((u - 1, emit_reduce))
                # mirror colsums, padded to MIRW with dummy repeats of pos 0.
                # The strip's scratch is the consumed head of the C arena
                # (WAR on the chunk-2 consumer orders it safely).
                cols = [pos for pos, d in enumerate(dlist) if d != 0]
                cols = cols + [0] * (MIRW - len(cols))
                o = ones if kind == "act" else ones_f

                def emit_mirror(xpc=xpc, ci=ci, cols=cols, ef=ef, o=o,
                                scr0=scr0):
                    for k, pos in enumerate(cols):
                        col = scr0 + ci * MIRW + k
                        nc.tensor.matmul(
                            xpc[:, col:col + 1],
                            ef[:, pos * P:(pos + 1) * P],
                            o[:], start=True, stop=True,
                        )

                pending.append((u - 1, emit_mirror))
                flush_pending(u - 4)
            sbase = (h * 8 + si) * NCHUNK * MIRW

            def emit_copy(xpc=xpc, sbase=sbase, scr0=scr0):
                nc.vector.tensor_copy(
                    marc[:, sbase:sbase + NCHUNK * MIRW],
                    xpc[:, scr0:scr0 + NCHUNK * MIRW],
                )

            pending.append((u - 1, emit_copy))
            if h == 1 and si == 7:
                # ship everything already final (strips 0..14 + batched
                # units) so only a sliver of output DMA trails the last ACT
                nc.scalar.dma_start(mirror_ext[:, :540], marc[:, :540])
                nc.scalar.dma_start(mirror_ext[:, 576:], marc[:, 576:])
                nc.sync.dma_start(slots_ext[:, :45], slots[:, :45])
                nc.sync.dma_start(slots_ext[:, 48:], slots[:, 48:])
            if si == 1:
                emit_batched(h, [(b, (b + 31) % NB) for b in STRIP_I0[:4]],
                             SLOT_D31[h], MARC_D31[h])
            elif si == 3:
                emit_batched(h, [(b, (b + 31) % NB) for b in STRIP_I0[4:]],
                             SLOT_D31[h] + 4, MARC_D31[h] + 4)
            elif si == 5:
                emit_batched(h, [(b, b + 32) for b in D32_I0],
                             SLOT_D32[h], MARC_D32[h])

    flush_pending(10 ** 9)
    nc.sync.dma_start(slots_ext[:, 45:48], slots[:, 45:48])
    nc.scalar.dma_start(mirror_ext[:, 540:576], marc[:, 540:576])


def build_kernel() -> bass.Bass:
    nc = bacc.Bacc("TRN2", target_bir_lowering=False, debug=False,
                   num_devices=NCORES)
    zt_ext = nc.dram_tensor("zt", (D, M), BF16, kind="ExternalInput").ap()
    slots_ext = nc.dram_tensor("slots", (P, NSLOTS), F32,
                               kind="ExternalOutput").ap()
    mirror_ext = nc.dram_tensor("mirror", (P, NSLOTS * MIRW), F32,
                                kind="ExternalOutput").ap()
    with tile.TileContext(nc) as tc:
        with ExitStack() as ctx:
            _build(ctx, tc, zt_ext, slots_ext, mirror_ext)
    nc.compile()
    return nc


_CACHE: dict = {}


def _normalize_bf16(z):
    """Host prep: f64 row-normalize then bf16 round (returns f32 values)."""
    import ml_dtypes

    zf = np.asarray(z, dtype=np.float64)
    zf = zf / np.maximum(np.linalg.norm(zf, axis=1, keepdims=True), 1e-12)
    return zf.astype(np.float32).astype(ml_dtypes.bfloat16)


def host_reduce(z, slots_all, mirror_all):
    """Combine per-core partials into the scalar loss (host, O(M) work)."""
    z = np.asarray(z, dtype=np.float32)
    mmap = mirror_map()
    S = np.zeros((2, HALF), dtype=np.float64)      # row sums incl. self term
    for c in range(NCORES):
        slots = slots_all[c].astype(np.float64).T    # (NSLOTS, P)
        mirror = mirror_all[c].astype(np.float64).T  # (NSLOTS*MIRW, P)
        for h in range(2):
            for si, i0 in enumerate(STRIP_I0):
                gb = (i0 + c) % NB
                rows = slice(gb * P, (gb + 1) * P)
                base = (h * 8 + si) * NCHUNK
                S[h, rows] += slots[base:base + NCHUNK].sum(axis=0)
            for idx, i0 in enumerate(STRIP_I0):
                gb = (i0 + c) % NB
                rows = slice(gb * P, (gb + 1) * P)
                S[h, rows] += slots[SLOT_D31[h] + idx]
            for idx, i0 in enumerate(D32_I0):
                gb = (i0 + c) % NB
                rows = slice(gb * P, (gb + 1) * P)
                S[h, rows] += slots[SLOT_D32[h] + idx]
        for col, h, bj in mmap:
            gb = (bj + c) % NB
            S[h, gb * P:(gb + 1) * P] += mirror[col]

    # self-term replica: exp(||zt_i||^2 / tau) from the same bf16 values the
    # device matmuls consume
    zt = _normalize_bf16(z).astype(np.float64)
    selfexp = np.exp((zt ** 2).sum(axis=1) * SCALE)
    Sa = S[0] - selfexp[:HALF]
    Sp = S[1] - selfexp[HALF:]

    # pdot from the true f32 inputs (exact math; device never computes it)
    zf = z.astype(np.float64)
    zf = zf / np.maximum(np.linalg.norm(zf, axis=1, keepdims=True), 1e-12)
    pdot = np.sum(zf[:HALF] * zf[HALF:], axis=1)

    terms = np.log(Sa) + np.log(Sp) - pdot * SCALE
    return np.float32(terms.mean())


def kernel(z, _trace: bool = False):
    z = np.ascontiguousarray(np.asarray(z, dtype=np.float32))
    assert z.shape == (M, D), z.shape
    if "nc" not in _CACHE:
        _CACHE["nc"] = build_kernel()
    nc = _CACHE["nc"]

    zt = _normalize_bf16(z)
    za, zp = zt[:HALF], zt[HALF:]
    in_maps = []
    for c in range(NCORES):
        zrot = np.concatenate(
            [np.roll(za, -P * c, axis=0), np.roll(zp, -P * c, axis=0)], axis=0
        )
        in_maps.append({"zt": np.ascontiguousarray(zrot.T)})

    res = run_bass_kernel_spmd(
        nc, in_maps, core_ids=list(range(NCORES)), trace=_trace
    )
    _CACHE["last_results"] = res
    slots_all = [r["slots"] for r in res.results]
    mirror_all = [r["mirror"] for r in res.results]
    return host_reduce(z, slots_all, mirror_all)
